# revision 73
# baseline (speedup 1.0000x reference)
"""Binarized DenseNet (nn_DenseNet_5841155522939) Trainium2 Bass kernel.

Strategy (data-parallel, 8 cores x 32 samples, processed as 16 sample-PAIRS):
  - Post-layer-1 activations and conv weights are exactly +-1: everything runs
    in fp8e4 with exact integer accumulation in fp32 PSUM.
  - Canvas layout per pair: [128 partitions = 2 samples x 64 ch, 66x66 cells]
    with a full duplicate at free offset 4367, so a +1-column tap partner sits
    at pair-stride 4368 (16B-aligned, required by DoubleRow).
  - conv2..conv6 use fp8 DoubleRow matmuls (2 taps per pair dim, 0.5 cyc/row):
    5 matmuls per 512-pixel chunk cover all 9 taps for BOTH samples (M=128
    block-diagonal), K_eff = 256.
  - BN+sign: each PSUM half-canvas tile is signed by TWO engines in parallel
    (ScalarE Sign on rows 1-24, DVE is_ge+decode on rows 25-32) so the sign
    latency always hides under the next tile's matmul fill and the 2-buffer
    PSUM ping-pong never stalls the PE.
  - Canvases are 8 persistent SBUF tiles; pad cells are memset once at start
    and never touched again (signs write interior only, the dup copy
    propagates pads), so no per-layer memsets exist.
  - The canvas duplicate is ONE whole-canvas DMA per layer output, issued from
    the Sync engine one conv-call after the signs (its wait is then already
    satisfied), halving HWDGE holds vs per-half copies.
  - conv6 output is stored as {0,1} (single DVE is_ge); the FC layer absorbs
    the 2u-1 decode into a folded bias and a final x2 scale.
  - Layer 1 (cin=1, stride 2, f32 input): exact 3-way bf16 split of x done on
    GpSimd directly into the parity-rearranged layout (no DVE traffic), one
    K=54 matmul per chunk (9 taps x 3 splits x 2 samples, block-diagonal),
    staged via a parity-split DRAM roundtrip (one gather DMA per tap pair).
  - FC: 256 DoubleRow matmuls, K=256 features each, N=32 samples.
"""

import numpy as np
from contextlib import ExitStack

import concourse.bacc as bacc
import concourse.bass as bass
import concourse.tile as tile
from concourse import mybir
from concourse.bass_utils import run_bass_kernel_spmd

FP8 = mybir.dt.float8e4
BF16 = mybir.dt.bfloat16
F32 = mybir.dt.float32
NP_FP8 = mybir.dt.np(FP8)
NP_BF16 = mybir.dt.np(BF16)
DR = mybir.MatmulPerfMode.DoubleRow
ALU = mybir.AluOpType
SIGN = mybir.ActivationFunctionType.Sign

B = 256
NCORES = 8
BPC = B // NCORES          # 32 samples per core
NPAIR = BPC // 2           # 16 pairs per core
NCH = 64
WP = 66                    # canvas row pitch
S1 = WP * WP               # 4356 cells per canvas
DUPO = 4367                # duplicate canvas base (DUPO % 16 == 15)
P1 = DUPO + 1              # pair stride hitting canvas[o+1] (4368, 16-aligned)
CSZ = 8736                 # canvas tile free size
EPS = np.float32(1e-5)
NSE = 12                   # interior rows per psum quarter on ScalarE (of 16)

# conv2-5 matmuls: (offset rel. to interior pixel p, pair stride, lo tap, hi tap)
# tap (r, c) = kernel index; cell = p + (r-1)*66 + (c-1)
MM5 = [
    (-67, P1, (0, 0), (0, 1)),
    (-65, 64, (0, 2), (1, 0)),
    (0,   P1, (1, 1), (1, 2)),
    (65,  P1, (2, 0), (2, 1)),
    (67,  64, (2, 2), None),
]
# conv6 (stride 2): base b6 = 2h'*66 + 2w'; tap (r, c) at b6 + r*66 + c
MM6 = [
    (0,   P1, (0, 0), (0, 1)),
    (2,   64, (0, 2), (1, 0)),
    (67,  P1, (1, 1), (1, 2)),
    (132, P1, (2, 0), (2, 1)),
    (134, 64, (2, 2), None),
]


def _thresholds(g, b, m, v, cmax=600):
    """Per-channel (scale, bias) s.t. Sign(scale*c + bias) == reference
    sign((c-m)*g*rsqrt(v+eps)+b) for every integer c in [-cmax, cmax]."""
    inv = (np.float32(1.0) / np.sqrt((v + EPS).astype(np.float32))).astype(np.float32)
    s = (g * inv).astype(np.float32)
    C = np.arange(-cmax, cmax + 1, dtype=np.float32)
    P = len(g)
    scale = np.zeros(P, np.float32)
    bias = np.zeros(P, np.float32)
    for c in range(P):
        vals = ((C - m[c]) * s[c] + b[c]).astype(np.float32)
        sg = np.sign(vals)
        if np.any(sg == 0.0):
            raise RuntimeError(f"exact-zero BN output, channel {c}")
        if np.all(sg == sg[0]):
            scale[c] = 0.0
            bias[c] = sg[0]
            continue
        d = np.diff(sg)
        idx = np.nonzero(d)[0]
        if len(idx) != 1:
            raise RuntimeError(f"non-monotone BN sign, channel {c}")
        T = C[idx[0] + 1]
        if sg[0] < 0:
            scale[c] = 1.0
            bias[c] = np.float32(-(T - 0.5))
        else:
            scale[c] = -1.0
            bias[c] = np.float32(T - 0.5)
    return scale, bias


def _sap(t, prow, pcount, off, dims):
    """AP into tile view `t` ([:] view): partition rows [prow, prow+pcount),
    free offset `off` elements, free dims [[step, count], ...]."""
    ps = t.ap[0][0]
    return bass.AP(tensor=t.tensor, offset=t.offset + prow * ps + off,
                   ap=[[ps, pcount]] + dims)


def _build_nc():
    nc = bacc.Bacc("TRN2", target_bir_lowering=False, debug=False,
                   num_devices=NCORES)
    # ds: host-staged exact 3-way bf16 split of x, parity-rearranged:
    # ds[pair, h, b*384 + px*192 + s*64 + w'] = split_s(x)[b, h, 2w'+px]
    d_ds = nc.dram_tensor("ds", [NPAIR, 128, 768], BF16, kind="ExternalInput")
    d_z = nc.dram_tensor("zrow", [5, 4096], BF16, kind="ExternalInput")
    d_w1 = nc.dram_tensor("w1s", [64, 128], BF16, kind="ExternalInput")
    d_wc = nc.dram_tensor("wc", [128, 5, 5, 2, 128], FP8, kind="ExternalInput")
    d_sbb = nc.dram_tensor("sbb", [128, 4, 6], F32, kind="ExternalInput")
    d_wfc = nc.dram_tensor("wfc_l", [128, 256, 2, 16], FP8, kind="ExternalInput")
    d_bfc = nc.dram_tensor("bfc_t", [12, 1], F32, kind="ExternalInput")
    d_out = nc.dram_tensor("out", [BPC, 12], F32, kind="ExternalOutput")

    with tile.TileContext(nc) as tc, ExitStack() as ctx:
        constp = ctx.enter_context(tc.tile_pool(name="const", bufs=1))
        tmpp = ctx.enter_context(tc.tile_pool(name="tmp", bufs=3))
        psump = ctx.enter_context(tc.tile_pool(name="psum", bufs=4, space="PSUM"))

        # ---- constants
        w1 = constp.tile([64, 128], BF16)
        wc = constp.tile([128, 5, 5, 2, 128], FP8)
        sbb = constp.tile([128, 4, 6], F32)   # rows 0-1: Sign form, 2-3: is_ge
        wfc = constp.tile([128, 256, 2, 16], FP8)
        bfc = constp.tile([12, 1], F32)
        act6 = constp.tile([128, BPC * 512], FP8)

        # ---- 8 persistent canvases; pad cells zeroed once, kept forever.
        # (signs write only interior cells; the dup DMA copies the whole main
        # region incl. pads, so dup pads stay zero too; gap+tail zeroed here.)
        canvases = [constp.tile([128, CSZ], FP8, tag=f"cnv{i}",
                                name=f"cnv{i}")
                    for i in range(8)]

        def init_canvas_pads():
            # top row; bottom row + gap; tail; side cols. The dup-region
            # interior is NOT zeroed: every read of it is preceded by a dup
            # DMA that fully overwrites [DUPO, DUPO+S1).
            for ci in (0, 5, 2, 7, 1, 6, 3, 4):   # order of first write
                t = canvases[ci][:]
                nc.vector.memset(_sap(t, 0, 128, 0, [[1, WP]]), 0.0)
                nc.vector.memset(_sap(t, 0, 128, 65 * WP,
                                      [[1, DUPO - 65 * WP]]), 0.0)
                nc.vector.memset(_sap(t, 0, 128, DUPO + S1,
                                      [[1, CSZ - DUPO - S1]]), 0.0)
                nc.vector.memset(_sap(t, 0, 128, 0, [[WP, WP], [65, 2]]), 0.0)

        def sign_psum(ps, cnv, q4, li):
            """BN+sign of a quarter psum tile [128, 1024] (interior rows
            16*q4+1..+16) into the canvas, split across ScalarE (first NSE
            rows) and DVE (rest): the sign latency always hides under ~1
            matmul-fill window, and with 4 PSUM tiles in rotation even the
            short conv6/conv1 boundary calls never stall on a draining
            sign."""
            base = (16 * q4 + 1) * WP + 1
            in_s = _sap(ps[:], 0, 128, 0, [[64, NSE], [1, 64]])
            out_s = _sap(cnv[:], 0, 128, base, [[WP, NSE], [1, 64]])
            nc.scalar.activation(out_s, in_s, SIGN,
                                 bias=sbb[:, 1, li:li + 1],
                                 scale=sbb[:, 0, li:li + 1])
            nv = 16 - NSE
            in_v = _sap(ps[:], 0, 128, 64 * NSE, [[64, nv], [1, 64]])
            tmp = tmpp.tile([128, 256], BF16, tag="sgtmp")
            tin = _sap(tmp[:], 0, 128, 0, [[64, nv], [1, 64]])
            nc.vector.tensor_scalar(
                tin, in_v, sbb[:, 2, li:li + 1], sbb[:, 3, li:li + 1],
                ALU.mult, ALU.is_ge)
            out_v = _sap(cnv[:], 0, 128, base + NSE * WP, [[WP, nv], [1, 64]])
            nc.vector.tensor_scalar(out_v, tin, 2.0, -1.0, ALU.mult, ALU.add)

        # Deferred DMA issue: closures run at the start of a LATER conv call,
        # so the issuing engine sees its dependencies already (nearly)
        # satisfied instead of blocking its sequencer on a wait. Front
        # gather-batches are rationed to one per conv call to keep the Sync
        # sequencer from backing up.
        pend_dups = []
        pend_misc = []

        def service_dups():
            while pend_dups:
                t = pend_dups.pop(0)[:]
                nc.sync.dma_start(out=_sap(t, 0, 128, DUPO, [[1, S1]]),
                                  in_=_sap(t, 0, 128, 0, [[1, S1]]))
            while pend_misc:
                pend_misc.pop(0)()

        # 6 persistent X27 buffers, partition layout 32*b + 3*tap + split
        # (the PE's 32-row accumulation grouping keeps per-sample f32 sums
        # bit-identical to a K=27 matmul): the h'=0 row, w'=0 col and the
        # zero-weight K rows 27-31 are initialized once and never re-written
        # by the gathers.
        x27s = [constp.tile([64, 4096], BF16, tag=f"x27_{i}", name=f"x27_{i}")
                for i in range(6)]

        def init_x27s():
            for i, X27 in enumerate(x27s):
                nc.vector.memset(_sap(X27[:], 0, 64, 0, [[1, 64]]), 0.0)
                nc.vector.memset(_sap(X27[:], 0, 64, 0, [[64, 64], [1, 1]]),
                                 0.0)

        def zrow(i, eng):
            eng.dma_start(out=_sap(x27s[i][:], 27, 5, 0, [[1, 4096]]),
                          in_=d_z[:])

        def emit_front(pp, startup=False):
            """Layer-1 input pipeline for pair pp: the exact bf16 split of x
            is staged on the HOST (ds input), so the whole front is just 12
            gather DMAs straight from DRAM, split Sync/GpSimd (startup:
            Sync/ScalarE/GpSimd to minimize the first conv1's latency)."""
            X27 = x27s[pp % 6]
            dso = d_ds[:].offset + pp * 128 * 768
            gi = 0
            for taps in ((0,), (1, 2), (3,), (4, 5), (6,), (7, 8)):
                t0 = taps[0]
                dy, dx = t0 // 3, t0 % 3
                pxx = (dx - 1) % 2
                rw = (dx - 1 - pxx) // 2
                h0 = 1 if dy == 0 else 0
                w0 = -rw
                ch, cw = 64 - h0, 64 - w0
                r0 = 2 * h0 + dy - 1   # first input row used; stride 2 rows
                nt = len(taps)
                lead = [[192, 2]] if nt == 2 else []
                for bb in range(2):
                    srcp = bass.AP(
                        tensor=d_ds[:].tensor,
                        offset=(dso + r0 * 768 + bb * 384 + pxx * 192),
                        ap=lead + [[64, 3], [1536, ch], [1, cw]])
                    dst = _sap(X27[:], 32 * bb + 3 * t0, 3 * nt,
                               h0 * 64 + w0, [[64, ch], [1, cw]])
                    if startup:
                        eng = (nc.sync, nc.scalar, nc.gpsimd)[gi % 3]
                    else:
                        eng = (nc.sync, nc.gpsimd)[gi % 2]
                    gi += 1
                    eng.dma_start(out=dst, in_=srcp)
            return X27

        def conv1(X27, cur):
            service_dups()
            for q4 in range(4):
                ps = psump.tile([128, 1024], F32, tag="ps")
                for qq in range(2):
                    q = 2 * q4 + qq
                    rhs = _sap(X27[:], 0, 59, q * 512, [[64, 8], [1, 64]])
                    nc.tensor.matmul(ps[:, 512 * qq:512 * qq + 512],
                                     lhsT=_sap(w1[:], 0, 59, 0, [[1, 128]]),
                                     rhs=rhs, start=True, stop=True)
                sign_psum(ps, cur, q4, 0)
            pend_dups.append(cur)

        def conv_mid(li, cur, nxt):
            service_dups()
            for q4 in range(4):
                ps = psump.tile([128, 1024], F32, tag="ps")
                for qq in range(2):
                    q = 2 * q4 + qq
                    base = (8 * q + 1) * WP + 1
                    for i, (o, P, _, _) in enumerate(MM5):
                        rhs = _sap(cur[:], 0, 128, base + o,
                                   [[P, 2], [WP, 8], [1, 64]])
                        nc.tensor.matmul(
                            ps[:, 512 * qq:512 * qq + 512],
                            lhsT=wc[:, li - 1, i], rhs=rhs,
                            start=(i == 0), stop=(i == 4), perf_mode=DR)
                sign_psum(ps, nxt, q4, li)
            pend_dups.append(nxt)

        def conv6(cur, pp, last=False):
            service_dups()
            ps = psump.tile([128, 1024], F32, tag="ps")
            for h in range(2):
                for i, (o, P, _, _) in enumerate(MM6):
                    rhs = _sap(cur[:], 0, 128, h * 2112 + o,
                               [[P, 2], [132, 16], [2, 32]])
                    nc.tensor.matmul(ps[:, 512 * h:512 * h + 512],
                                     lhsT=wc[:, 4, i], rhs=rhs,
                                     start=(i == 0), stop=(i == 4), perf_mode=DR)
            # conv6 output stays +-1 (Sign) so the FC needs no bias fold;
            # split ScalarE/DVE like the other layers.
            stg6 = tmpp.tile([128, 1024], FP8, tag="stg6")
            nc.scalar.activation(stg6[:, 0:768], ps[:, 0:768], SIGN,
                                 bias=sbb[:, 1, 5:6], scale=sbb[:, 0, 5:6])
            tmp6 = tmpp.tile([128, 256], BF16, tag="sgtmp")
            nc.vector.tensor_scalar(tmp6[:], ps[:, 768:1024],
                                    sbb[:, 2, 5:6], sbb[:, 3, 5:6],
                                    ALU.mult, ALU.is_ge)
            nc.vector.tensor_scalar(stg6[:, 768:1024], tmp6[:], 2.0, -1.0,
                                    ALU.mult, ALU.add)
            # rearrange [2s x 64ch, 1024px] -> act6[64ch x 2px-half, sample*512]
            # 2 copies on GpSimd, 2 deferred to Sync so the Pool engine does
            # not back up at group boundaries; the last group goes immediate
            # on Sync/ScalarE so the FC is not left waiting on Pool.
            for s in range(2):
                for ph in range(2):
                    dst = _sap(act6[:], 64 * ph, 64, (2 * pp + s) * 512,
                               [[1, 512]])
                    srcp = _sap(stg6[:], 64 * s, 64, 512 * ph, [[1, 512]])
                    if last:
                        (nc.sync if ph == 0 else nc.scalar).dma_start(
                            out=dst, in_=srcp)
                    elif ph == 0:
                        nc.gpsimd.dma_start(out=dst, in_=srcp)
                    else:
                        pend_misc.append(
                            lambda d=dst, sr=srcp:
                            nc.sync.dma_start(out=d, in_=sr))

        # Pairs interleaved layer-by-layer in groups of GIL: while one
        # pair's sign latency drains, the PE runs the other pairs' matmuls
        # (PE executes strictly in program order).
        GIL = 4
        # Startup: x27 inits precede the gathers; the first fronts' gathers
        # go out immediately (their ds source is an input tensor, ready at
        # t=0); small constants interleave on ScalarE.
        nc.scalar.dma_start(out=w1[:], in_=d_w1[:])
        init_x27s()
        zrow(0, nc.scalar)
        fronts = {0: emit_front(0, startup=True)}
        nc.scalar.dma_start(out=sbb[:], in_=d_sbb[:])
        zrow(1, nc.scalar)
        fronts[1] = emit_front(1, startup=True)
        for p in range(2, min(GIL, NPAIR)):
            fronts[p] = emit_front(p)
        zrow(2, nc.sync)
        zrow(3, nc.sync)
        zrow(4, nc.gpsimd)
        zrow(5, nc.gpsimd)
        nc.scalar.dma_start(out=bfc[:], in_=d_bfc[:])
        nc.scalar.dma_start(out=wc[:], in_=d_wc[:])
        init_canvas_pads()

        def slot(p, li):
            return canvases[(5 * p + li) % 8]

        # Software-pipelined schedule: group g's conv6 calls interleave with
        # group g+1's conv1 calls so the short conv6 matmul bursts never have
        # to hide a whole dup-DMA latency on their own, and the PE never sees
        # a group seam.
        for idx in range(GIL):
            conv1(fronts.pop(idx), slot(idx, 0))
        for g in range(0, NPAIR, GIL):
            grp = [g + i for i in range(GIL)]
            for li in range(1, 5):
                for idx, p in enumerate(grp):
                    if li == 1 + idx // 2 and p + GIL < NPAIR:
                        fronts[p + GIL] = emit_front(p + GIL)
                    if g == 0 and li == 3 and idx == 0:
                        nc.scalar.dma_start(out=wfc[:], in_=d_wfc[:])
                    conv_mid(li, slot(p, li - 1), slot(p, li))
            for idx, p in enumerate(grp):
                conv6(slot(p, 4), p, last=(g + GIL >= NPAIR))
                if p + GIL < NPAIR:
                    conv1(fronts.pop(p + GIL), slot(p + GIL, 0))
        service_dups()

        # ---- fc: 256 DoubleRow matmuls, K=256 features, N=32 samples
        psf = psump.tile([12, 32], F32, tag="ps")
        for j in range(256):
            rhs = _sap(act6[:], 0, 128, j, [[256, 2], [512, 32]])
            lhsT = bass.AP(tensor=wfc[:].tensor,
                           offset=wfc[:].offset + j * 32,
                           ap=[[wfc[:].ap[0][0], 128], [16, 2], [1, 12]])
            nc.tensor.matmul(psf[:], lhsT=lhsT, rhs=rhs,
                             start=(j == 0), stop=(j == 255), perf_mode=DR)
        accf = constp.tile([12, 32], F32)
        nc.vector.tensor_scalar(accf[:], psf[:], 1.0, bfc[:],
                                ALU.mult, ALU.add)
        nc.sync.dma_start(
            out=bass.AP(tensor=d_out[:].tensor, offset=0,
                        ap=[[1, 12], [12, BPC]]),
            in_=accf[:])

    nc.compile()
    return nc


_NC_CACHE = {}


def _prep_const_inputs(inputs):
    out = {}
    # conv1 weights: [64, 128] bf16, block-diag: sample A rows 0-26 (PE row
    # group 0), sample B rows 32-58 (row group 1) -> per-sample sums use the
    # same 32-row accumulation grouping as a K=27 matmul
    w1b = np.sign(np.asarray(inputs["w1"], np.float32))  # [64, 1, 3, 3]
    w1s = np.zeros((64, 128), NP_BF16)
    for t9 in range(9):
        dy, dx = t9 // 3, t9 % 3
        for bb in range(2):
            for s3 in range(3):
                w1s[32 * bb + 3 * t9 + s3, 64 * bb:64 * bb + 64] = \
                    w1b[:, 0, dy, dx].astype(NP_BF16)
    out["w1s"] = w1s
    out["zrow"] = np.zeros((5, 4096), NP_BF16)
    # conv2-6 DoubleRow weights, block-diagonal over the 2 samples
    wcx = np.zeros((128, 5, 5, 2, 128), NP_FP8)
    for li in range(5):
        w = np.sign(np.asarray(inputs[f"w{li + 2}"], np.float32))  # [O, I, 3, 3]
        table = MM5 if li < 4 else MM6
        for i, (_, _, lo, hi) in enumerate(table):
            for j, tap in enumerate((lo, hi)):
                if tap is None:
                    continue
                blk = w[:, :, tap[0], tap[1]].T.astype(NP_FP8)  # [I, O]
                wcx[0:64, li, i, j, 0:64] = blk
                wcx[64:128, li, i, j, 64:128] = blk
    out["wc"] = wcx
    # scales/biases [128, 4, 6]: rows 0-1 Sign form (sb), rows 2-3 is_ge (sb2)
    sb = np.zeros((128, 2, 6), np.float32)
    sb2 = np.zeros((128, 2, 6), np.float32)
    g1, b1, m1, v1 = (np.asarray(inputs[k], np.float32) for k in
                      ("g1", "b1", "m1", "v1"))
    inv = (np.float32(1.0) / np.sqrt((v1 + EPS).astype(np.float32))).astype(np.float32)
    s1 = (g1 * inv).astype(np.float32)
    sb[:, 0, 0] = np.tile(s1, 2)
    sb[:, 1, 0] = np.tile((b1 - m1 * s1).astype(np.float32), 2)
    sb2[:, 0, 0] = sb[:, 0, 0]
    sb2[:, 1, 0] = -sb[:, 1, 0]
    for li in range(1, 6):
        g_, b_, m_, v_ = (np.asarray(inputs[f"{k}{li + 1}"], np.float32)
                          for k in ("g", "b", "m", "v"))
        sc, bi = _thresholds(g_, b_, m_, v_)
        sb[:, 0, li] = np.tile(sc, 2)
        sb[:, 1, li] = np.tile(bi, 2)
        sb2[:, 0, li] = np.tile(sc, 2)
        sb2[:, 1, li] = np.tile(-bi, 2)
    out["sbb"] = np.concatenate([sb, sb2], axis=1)
    # fc: lhsT [128 = ch + 64*ph, j, pair i, cls(12, padded 16)]
    wfc_s = np.sign(np.asarray(inputs["wfc"], np.float32))  # [12, 65536]
    wr = wfc_s.reshape(12, 64, 2, 2, 256)  # [cls, ch, ph, i, j]
    wl = np.zeros((128, 256, 2, 16), NP_FP8)
    for ph in range(2):
        # wl[ch + 64*ph, j, i, cls]
        wl[64 * ph:64 * ph + 64, :, :, 0:12] = \
            wr[:, :, ph, :, :].transpose(1, 3, 2, 0).astype(NP_FP8)
    out["wfc_l"] = wl
    bfc = np.asarray(inputs["bfc"], np.float32)
    out["bfc_t"] = bfc.reshape(12, 1).astype(np.float32)
    return out


def _make_ds(xs):
    """Host staging: exact 3-way bf16 split of x, parity-rearranged to
    ds[pair, h, (px, b, s, w')]. A lossless re-encoding (s0+s1+s2 == x in
    f32), so the device sums are bit-identical to summing x directly."""
    x = np.asarray(xs, np.float32).reshape(NPAIR, 2, 128, 128)
    s0 = x.astype(NP_BF16)
    r = x - s0.astype(np.float32)
    s1 = r.astype(NP_BF16)
    s2 = (r - s1.astype(np.float32)).astype(NP_BF16)
    sp = np.stack([s0, s1, s2], axis=0)        # [s, pair, b, h, w]
    sp = sp.reshape(3, NPAIR, 2, 128, 64, 2)   # [s, pair, b, h, w', px]
    ds = sp.transpose(1, 3, 2, 5, 0, 4)        # [pair, h, b, px, s, w']
    return np.ascontiguousarray(ds).reshape(NPAIR, 128, 768)


def kernel(**inputs):
    if "nc" not in _NC_CACHE:
        _NC_CACHE["nc"] = _build_nc()
    nc = _NC_CACHE["nc"]
    const = _prep_const_inputs(inputs)
    x = np.asarray(inputs["x"], np.float32)
    in_maps = []
    for c in range(NCORES):
        m = dict(const)
        m["ds"] = _make_ds(x[c * BPC:(c + 1) * BPC])
        in_maps.append(m)
    res = run_bass_kernel_spmd(nc, in_maps, core_ids=list(range(NCORES)))
    return np.concatenate([r["out"] for r in res.results], axis=0)


# revision 76
# speedup vs baseline: 1.0034x; 1.0034x over previous
"""Binarized DenseNet (nn_DenseNet_5841155522939) Trainium2 Bass kernel.

Strategy (data-parallel, 8 cores x 32 samples, processed as 16 sample-PAIRS):
  - Post-layer-1 activations and conv weights are exactly +-1: everything runs
    in fp8e4 with exact integer accumulation in fp32 PSUM.
  - Canvas layout per pair: [128 partitions = 2 samples x 64 ch, 66x66 cells]
    with a full duplicate at free offset 4367, so a +1-column tap partner sits
    at pair-stride 4368 (16B-aligned, required by DoubleRow).
  - conv2..conv6 use fp8 DoubleRow matmuls (2 taps per pair dim, 0.5 cyc/row):
    5 matmuls per 512-pixel chunk cover all 9 taps for BOTH samples (M=128
    block-diagonal), K_eff = 256.
  - BN+sign: each PSUM half-canvas tile is signed by TWO engines in parallel
    (ScalarE Sign on rows 1-24, DVE is_ge+decode on rows 25-32) so the sign
    latency always hides under the next tile's matmul fill and the 2-buffer
    PSUM ping-pong never stalls the PE.
  - Canvases are 8 persistent SBUF tiles; pad cells are memset once at start
    and never touched again (signs write interior only, the dup copy
    propagates pads), so no per-layer memsets exist.
  - The canvas duplicate is ONE whole-canvas DMA per layer output, issued from
    the Sync engine one conv-call after the signs (its wait is then already
    satisfied), halving HWDGE holds vs per-half copies.
  - conv6 output is stored as {0,1} (single DVE is_ge); the FC layer absorbs
    the 2u-1 decode into a folded bias and a final x2 scale.
  - Layer 1 (cin=1, stride 2, f32 input): exact 3-way bf16 split of x done on
    GpSimd directly into the parity-rearranged layout (no DVE traffic), one
    K=54 matmul per chunk (9 taps x 3 splits x 2 samples, block-diagonal),
    staged via a parity-split DRAM roundtrip (one gather DMA per tap pair).
  - FC: 256 DoubleRow matmuls, K=256 features each, N=32 samples.
"""

import numpy as np
from contextlib import ExitStack

import concourse.bacc as bacc
import concourse.bass as bass
import concourse.tile as tile
from concourse import mybir
from concourse.bass_utils import run_bass_kernel_spmd

FP8 = mybir.dt.float8e4
BF16 = mybir.dt.bfloat16
F32 = mybir.dt.float32
NP_FP8 = mybir.dt.np(FP8)
NP_BF16 = mybir.dt.np(BF16)
DR = mybir.MatmulPerfMode.DoubleRow
ALU = mybir.AluOpType
SIGN = mybir.ActivationFunctionType.Sign

B = 256
NCORES = 8
BPC = B // NCORES          # 32 samples per core
NPAIR = BPC // 2           # 16 pairs per core
NCH = 64
WP = 66                    # canvas row pitch
S1 = WP * WP               # 4356 cells per canvas
DUPO = 4367                # duplicate canvas base (DUPO % 16 == 15)
P1 = DUPO + 1              # pair stride hitting canvas[o+1] (4368, 16-aligned)
CSZ = 8736                 # canvas tile free size
EPS = np.float32(1e-5)
NSE = 12                   # interior rows per psum quarter on ScalarE (of 16)

# conv2-5 matmuls: (offset rel. to interior pixel p, pair stride, lo tap, hi tap)
# tap (r, c) = kernel index; cell = p + (r-1)*66 + (c-1)
MM5 = [
    (-67, P1, (0, 0), (0, 1)),
    (-65, 64, (0, 2), (1, 0)),
    (0,   P1, (1, 1), (1, 2)),
    (65,  P1, (2, 0), (2, 1)),
    (67,  64, (2, 2), None),
]
# conv6 (stride 2): base b6 = 2h'*66 + 2w'; tap (r, c) at b6 + r*66 + c
MM6 = [
    (0,   P1, (0, 0), (0, 1)),
    (2,   64, (0, 2), (1, 0)),
    (67,  P1, (1, 1), (1, 2)),
    (132, P1, (2, 0), (2, 1)),
    (134, 64, (2, 2), None),
]


def _thresholds(g, b, m, v, cmax=600):
    """Per-channel (scale, bias) s.t. Sign(scale*c + bias) == reference
    sign((c-m)*g*rsqrt(v+eps)+b) for every integer c in [-cmax, cmax]."""
    inv = (np.float32(1.0) / np.sqrt((v + EPS).astype(np.float32))).astype(np.float32)
    s = (g * inv).astype(np.float32)
    C = np.arange(-cmax, cmax + 1, dtype=np.float32)
    P = len(g)
    scale = np.zeros(P, np.float32)
    bias = np.zeros(P, np.float32)
    for c in range(P):
        vals = ((C - m[c]) * s[c] + b[c]).astype(np.float32)
        sg = np.sign(vals)
        if np.any(sg == 0.0):
            raise RuntimeError(f"exact-zero BN output, channel {c}")
        if np.all(sg == sg[0]):
            scale[c] = 0.0
            bias[c] = sg[0]
            continue
        d = np.diff(sg)
        idx = np.nonzero(d)[0]
        if len(idx) != 1:
            raise RuntimeError(f"non-monotone BN sign, channel {c}")
        T = C[idx[0] + 1]
        if sg[0] < 0:
            scale[c] = 1.0
            bias[c] = np.float32(-(T - 0.5))
        else:
            scale[c] = -1.0
            bias[c] = np.float32(T - 0.5)
    return scale, bias


def _sap(t, prow, pcount, off, dims):
    """AP into tile view `t` ([:] view): partition rows [prow, prow+pcount),
    free offset `off` elements, free dims [[step, count], ...]."""
    ps = t.ap[0][0]
    return bass.AP(tensor=t.tensor, offset=t.offset + prow * ps + off,
                   ap=[[ps, pcount]] + dims)


def _build_nc():
    nc = bacc.Bacc("TRN2", target_bir_lowering=False, debug=False,
                   num_devices=NCORES)
    # ds: host-staged exact 3-way bf16 split of x, parity-rearranged:
    # ds[pair, h, b*384 + px*192 + s*64 + w'] = split_s(x)[b, h, 2w'+px]
    d_ds = nc.dram_tensor("ds", [NPAIR, 128, 768], BF16, kind="ExternalInput")
    d_z = nc.dram_tensor("zrow", [5, 4096], BF16, kind="ExternalInput")
    d_w1 = nc.dram_tensor("w1s", [64, 128], BF16, kind="ExternalInput")
    d_wc = nc.dram_tensor("wc", [128, 5, 5, 2, 128], FP8, kind="ExternalInput")
    d_sbb = nc.dram_tensor("sbb", [128, 4, 6], F32, kind="ExternalInput")
    d_wfc = nc.dram_tensor("wfc_l", [128, 256, 2, 16], FP8, kind="ExternalInput")
    d_bfc = nc.dram_tensor("bfc_t", [12, 1], F32, kind="ExternalInput")
    d_out = nc.dram_tensor("out", [BPC, 12], F32, kind="ExternalOutput")

    with tile.TileContext(nc) as tc, ExitStack() as ctx:
        constp = ctx.enter_context(tc.tile_pool(name="const", bufs=1))
        tmpp = ctx.enter_context(tc.tile_pool(name="tmp", bufs=3))
        psump = ctx.enter_context(tc.tile_pool(name="psum", bufs=4, space="PSUM"))

        # ---- constants
        w1 = constp.tile([64, 128], BF16)
        wc = constp.tile([128, 5, 5, 2, 128], FP8)
        sbb = constp.tile([128, 4, 6], F32)   # rows 0-1: Sign form, 2-3: is_ge
        wfc = constp.tile([128, 256, 2, 16], FP8)
        bfc = constp.tile([12, 1], F32)
        act6 = constp.tile([128, BPC * 512], FP8)

        # ---- 8 persistent canvases; pad cells zeroed once, kept forever.
        # (signs write only interior cells; the dup DMA copies the whole main
        # region incl. pads, so dup pads stay zero too; gap+tail zeroed here.)
        canvases = [constp.tile([128, CSZ], FP8, tag=f"cnv{i}",
                                name=f"cnv{i}")
                    for i in range(8)]

        def init_canvas_pads():
            # top row; bottom row + gap; tail; side cols. The dup-region
            # interior is NOT zeroed: every read of it is preceded by a dup
            # DMA that fully overwrites [DUPO, DUPO+S1).
            for ci in (0, 5, 2, 7, 1, 6, 3, 4):   # order of first write
                t = canvases[ci][:]
                nc.vector.memset(_sap(t, 0, 128, 0, [[1, WP]]), 0.0)
                nc.vector.memset(_sap(t, 0, 128, 65 * WP,
                                      [[1, DUPO - 65 * WP]]), 0.0)
                nc.vector.memset(_sap(t, 0, 128, DUPO + S1,
                                      [[1, CSZ - DUPO - S1]]), 0.0)
                nc.vector.memset(_sap(t, 0, 128, 0, [[WP, WP], [65, 2]]), 0.0)

        def sign_psum(ps, cnv, q4, li):
            """BN+sign of a quarter psum tile [128, 1024] (interior rows
            16*q4+1..+16) into the canvas, split across ScalarE (first NSE
            rows) and DVE (rest): the sign latency always hides under ~1
            matmul-fill window, and with 4 PSUM tiles in rotation even the
            short conv6/conv1 boundary calls never stall on a draining
            sign."""
            base = (16 * q4 + 1) * WP + 1
            in_s = _sap(ps[:], 0, 128, 0, [[64, NSE], [1, 64]])
            out_s = _sap(cnv[:], 0, 128, base, [[WP, NSE], [1, 64]])
            nc.scalar.activation(out_s, in_s, SIGN,
                                 bias=sbb[:, 1, li:li + 1],
                                 scale=sbb[:, 0, li:li + 1])
            nv = 16 - NSE
            in_v = _sap(ps[:], 0, 128, 64 * NSE, [[64, nv], [1, 64]])
            tmp = tmpp.tile([128, 256], BF16, tag="sgtmp")
            tin = _sap(tmp[:], 0, 128, 0, [[64, nv], [1, 64]])
            nc.vector.tensor_scalar(
                tin, in_v, sbb[:, 2, li:li + 1], sbb[:, 3, li:li + 1],
                ALU.mult, ALU.is_ge)
            out_v = _sap(cnv[:], 0, 128, base + NSE * WP, [[WP, nv], [1, 64]])
            nc.vector.tensor_scalar(out_v, tin, 2.0, -1.0, ALU.mult, ALU.add)

        # Deferred DMA issue: closures run at the start of a LATER conv call,
        # so the issuing engine sees its dependencies already (nearly)
        # satisfied instead of blocking its sequencer on a wait. Front
        # gather-batches are rationed to one per conv call to keep the Sync
        # sequencer from backing up.
        pend_dups = []
        pend_misc = []

        def service_dups():
            while pend_dups:
                t = pend_dups.pop(0)[:]
                nc.sync.dma_start(out=_sap(t, 0, 128, DUPO, [[1, S1]]),
                                  in_=_sap(t, 0, 128, 0, [[1, S1]]))
            while pend_misc:
                pend_misc.pop(0)()

        # 6 persistent X27 buffers, partition layout 32*b + 3*tap + split
        # (the PE's 32-row accumulation grouping keeps per-sample f32 sums
        # bit-identical to a K=27 matmul): the h'=0 row, w'=0 col and the
        # zero-weight K rows 27-31 are initialized once and never re-written
        # by the gathers.
        x27s = [constp.tile([64, 4096], BF16, tag=f"x27_{i}", name=f"x27_{i}")
                for i in range(6)]

        def init_x27s():
            for i, X27 in enumerate(x27s):
                nc.vector.memset(_sap(X27[:], 0, 64, 0, [[1, 64]]), 0.0)
                nc.vector.memset(_sap(X27[:], 0, 64, 0, [[64, 64], [1, 1]]),
                                 0.0)

        def zrow(i, eng):
            eng.dma_start(out=_sap(x27s[i][:], 27, 5, 0, [[1, 4096]]),
                          in_=d_z[:])

        def emit_front(pp, startup=False):
            """Layer-1 input pipeline for pair pp: the exact bf16 split of x
            is staged on the HOST (ds input), so the whole front is just 12
            gather DMAs straight from DRAM, split Sync/GpSimd (startup:
            Sync/ScalarE/GpSimd to minimize the first conv1's latency)."""
            X27 = x27s[pp % 6]
            dso = d_ds[:].offset + pp * 128 * 768
            gi = 0
            for taps in ((0,), (1, 2), (3,), (4, 5), (6,), (7, 8)):
                t0 = taps[0]
                dy, dx = t0 // 3, t0 % 3
                pxx = (dx - 1) % 2
                rw = (dx - 1 - pxx) // 2
                h0 = 1 if dy == 0 else 0
                w0 = -rw
                ch, cw = 64 - h0, 64 - w0
                r0 = 2 * h0 + dy - 1   # first input row used; stride 2 rows
                nt = len(taps)
                lead = [[192, 2]] if nt == 2 else []
                for bb in range(2):
                    srcp = bass.AP(
                        tensor=d_ds[:].tensor,
                        offset=(dso + r0 * 768 + bb * 384 + pxx * 192),
                        ap=lead + [[64, 3], [1536, ch], [1, cw]])
                    dst = _sap(X27[:], 32 * bb + 3 * t0, 3 * nt,
                               h0 * 64 + w0, [[64, ch], [1, cw]])
                    if startup == 1:
                        eng = (nc.sync, nc.scalar, nc.gpsimd)[gi % 3]
                    elif startup == 2:
                        eng = (nc.sync, nc.sync, nc.gpsimd)[gi % 3]
                    else:
                        eng = (nc.sync, nc.gpsimd)[gi % 2]
                    gi += 1
                    eng.dma_start(out=dst, in_=srcp)
            return X27

        def conv1(X27, cur):
            service_dups()
            for q4 in range(4):
                ps = psump.tile([128, 1024], F32, tag="ps")
                for qq in range(2):
                    q = 2 * q4 + qq
                    rhs = _sap(X27[:], 0, 59, q * 512, [[64, 8], [1, 64]])
                    nc.tensor.matmul(ps[:, 512 * qq:512 * qq + 512],
                                     lhsT=_sap(w1[:], 0, 59, 0, [[1, 128]]),
                                     rhs=rhs, start=True, stop=True)
                sign_psum(ps, cur, q4, 0)
            pend_dups.append(cur)

        def conv_mid(li, cur, nxt):
            service_dups()
            for q4 in range(4):
                ps = psump.tile([128, 1024], F32, tag="ps")
                for qq in range(2):
                    q = 2 * q4 + qq
                    base = (8 * q + 1) * WP + 1
                    for i, (o, P, _, _) in enumerate(MM5):
                        rhs = _sap(cur[:], 0, 128, base + o,
                                   [[P, 2], [WP, 8], [1, 64]])
                        nc.tensor.matmul(
                            ps[:, 512 * qq:512 * qq + 512],
                            lhsT=wc[:, li - 1, i], rhs=rhs,
                            start=(i == 0), stop=(i == 4), perf_mode=DR)
                sign_psum(ps, nxt, q4, li)
            pend_dups.append(nxt)

        def conv6(cur, pp, last=False):
            service_dups()
            ps = psump.tile([128, 1024], F32, tag="ps")
            for h in range(2):
                for i, (o, P, _, _) in enumerate(MM6):
                    rhs = _sap(cur[:], 0, 128, h * 2112 + o,
                               [[P, 2], [132, 16], [2, 32]])
                    nc.tensor.matmul(ps[:, 512 * h:512 * h + 512],
                                     lhsT=wc[:, 4, i], rhs=rhs,
                                     start=(i == 0), stop=(i == 4), perf_mode=DR)
            # conv6 output stays +-1 (Sign) so the FC needs no bias fold;
            # split ScalarE/DVE like the other layers.
            stg6 = tmpp.tile([128, 1024], FP8, tag="stg6")
            nc.scalar.activation(stg6[:, 0:768], ps[:, 0:768], SIGN,
                                 bias=sbb[:, 1, 5:6], scale=sbb[:, 0, 5:6])
            tmp6 = tmpp.tile([128, 256], BF16, tag="sgtmp")
            nc.vector.tensor_scalar(tmp6[:], ps[:, 768:1024],
                                    sbb[:, 2, 5:6], sbb[:, 3, 5:6],
                                    ALU.mult, ALU.is_ge)
            nc.vector.tensor_scalar(stg6[:, 768:1024], tmp6[:], 2.0, -1.0,
                                    ALU.mult, ALU.add)
            # rearrange [2s x 64ch, 1024px] -> act6[64ch x 2px-half, sample*512]
            # 2 copies on GpSimd, 2 deferred to Sync so the Pool engine does
            # not back up at group boundaries; the last group goes immediate
            # on Sync/ScalarE so the FC is not left waiting on Pool.
            for s in range(2):
                for ph in range(2):
                    dst = _sap(act6[:], 64 * ph, 64, (2 * pp + s) * 512,
                               [[1, 512]])
                    srcp = _sap(stg6[:], 64 * s, 64, 512 * ph, [[1, 512]])
                    if ph == 0:
                        nc.gpsimd.dma_start(out=dst, in_=srcp)
                    else:
                        pend_misc.append(
                            lambda d=dst, sr=srcp:
                            nc.sync.dma_start(out=d, in_=sr))

        # Pairs interleaved layer-by-layer in groups of GIL: while one
        # pair's sign latency drains, the PE runs the other pairs' matmuls
        # (PE executes strictly in program order).
        GIL = 4
        # Startup: x27 inits precede the gathers; the first fronts' gathers
        # go out immediately (their ds source is an input tensor, ready at
        # t=0); small constants interleave on ScalarE.
        nc.scalar.dma_start(out=w1[:], in_=d_w1[:])
        init_x27s()
        zrow(0, nc.scalar)
        fronts = {0: emit_front(0, startup=1)}
        nc.scalar.dma_start(out=sbb[:], in_=d_sbb[:])
        zrow(1, nc.scalar)
        fronts[1] = emit_front(1, startup=1)
        zrow(2, nc.sync)
        fronts[2] = emit_front(2, startup=2)
        zrow(3, nc.sync)
        fronts[3] = emit_front(3, startup=2)
        pend_misc.append(lambda: zrow(4, nc.gpsimd))
        pend_misc.append(lambda: zrow(5, nc.gpsimd))
        nc.scalar.dma_start(out=bfc[:], in_=d_bfc[:])
        nc.scalar.dma_start(out=wc[:], in_=d_wc[:])
        init_canvas_pads()

        def slot(p, li):
            return canvases[(5 * p + li) % 8]

        # Software-pipelined schedule: group g's conv6 calls interleave with
        # group g+1's conv1 calls so the short conv6 matmul bursts never have
        # to hide a whole dup-DMA latency on their own, and the PE never sees
        # a group seam.
        for idx in range(GIL):
            conv1(fronts.pop(idx), slot(idx, 0))
        for g in range(0, NPAIR, GIL):
            grp = [g + i for i in range(GIL)]
            for li in range(1, 5):
                for idx, p in enumerate(grp):
                    if li == 1 + idx // 2 and p + GIL < NPAIR:
                        fronts[p + GIL] = emit_front(p + GIL)
                    if g == 0 and li == 3 and idx == 0:
                        nc.scalar.dma_start(out=wfc[:], in_=d_wfc[:])
                    conv_mid(li, slot(p, li - 1), slot(p, li))
            for idx, p in enumerate(grp):
                conv6(slot(p, 4), p, last=(g + GIL >= NPAIR))
                if p + GIL < NPAIR:
                    conv1(fronts.pop(p + GIL), slot(p + GIL, 0))
        service_dups()

        # ---- fc: 256 DoubleRow matmuls, K=256 features, N=32 samples
        psf = psump.tile([12, 32], F32, tag="ps")
        for j in range(256):
            rhs = _sap(act6[:], 0, 128, j, [[256, 2], [512, 32]])
            lhsT = bass.AP(tensor=wfc[:].tensor,
                           offset=wfc[:].offset + j * 32,
                           ap=[[wfc[:].ap[0][0], 128], [16, 2], [1, 12]])
            nc.tensor.matmul(psf[:], lhsT=lhsT, rhs=rhs,
                             start=(j == 0), stop=(j == 255), perf_mode=DR)
        accf = constp.tile([12, 32], F32)
        nc.vector.tensor_scalar(accf[:], psf[:], 1.0, bfc[:],
                                ALU.mult, ALU.add)
        nc.sync.dma_start(
            out=bass.AP(tensor=d_out[:].tensor, offset=0,
                        ap=[[1, 12], [12, BPC]]),
            in_=accf[:])

    nc.compile()
    return nc


_NC_CACHE = {}


def _prep_const_inputs(inputs):
    out = {}
    # conv1 weights: [64, 128] bf16, block-diag: sample A rows 0-26 (PE row
    # group 0), sample B rows 32-58 (row group 1) -> per-sample sums use the
    # same 32-row accumulation grouping as a K=27 matmul
    w1b = np.sign(np.asarray(inputs["w1"], np.float32))  # [64, 1, 3, 3]
    w1s = np.zeros((64, 128), NP_BF16)
    for t9 in range(9):
        dy, dx = t9 // 3, t9 % 3
        for bb in range(2):
            for s3 in range(3):
                w1s[32 * bb + 3 * t9 + s3, 64 * bb:64 * bb + 64] = \
                    w1b[:, 0, dy, dx].astype(NP_BF16)
    out["w1s"] = w1s
    out["zrow"] = np.zeros((5, 4096), NP_BF16)
    # conv2-6 DoubleRow weights, block-diagonal over the 2 samples
    wcx = np.zeros((128, 5, 5, 2, 128), NP_FP8)
    for li in range(5):
        w = np.sign(np.asarray(inputs[f"w{li + 2}"], np.float32))  # [O, I, 3, 3]
        table = MM5 if li < 4 else MM6
        for i, (_, _, lo, hi) in enumerate(table):
            for j, tap in enumerate((lo, hi)):
                if tap is None:
                    continue
                blk = w[:, :, tap[0], tap[1]].T.astype(NP_FP8)  # [I, O]
                wcx[0:64, li, i, j, 0:64] = blk
                wcx[64:128, li, i, j, 64:128] = blk
    out["wc"] = wcx
    # scales/biases [128, 4, 6]: rows 0-1 Sign form (sb), rows 2-3 is_ge (sb2)
    sb = np.zeros((128, 2, 6), np.float32)
    sb2 = np.zeros((128, 2, 6), np.float32)
    g1, b1, m1, v1 = (np.asarray(inputs[k], np.float32) for k in
                      ("g1", "b1", "m1", "v1"))
    inv = (np.float32(1.0) / np.sqrt((v1 + EPS).astype(np.float32))).astype(np.float32)
    s1 = (g1 * inv).astype(np.float32)
    sb[:, 0, 0] = np.tile(s1, 2)
    sb[:, 1, 0] = np.tile((b1 - m1 * s1).astype(np.float32), 2)
    sb2[:, 0, 0] = sb[:, 0, 0]
    sb2[:, 1, 0] = -sb[:, 1, 0]
    for li in range(1, 6):
        g_, b_, m_, v_ = (np.asarray(inputs[f"{k}{li + 1}"], np.float32)
                          for k in ("g", "b", "m", "v"))
        sc, bi = _thresholds(g_, b_, m_, v_)
        sb[:, 0, li] = np.tile(sc, 2)
        sb[:, 1, li] = np.tile(bi, 2)
        sb2[:, 0, li] = np.tile(sc, 2)
        sb2[:, 1, li] = np.tile(-bi, 2)
    out["sbb"] = np.concatenate([sb, sb2], axis=1)
    # fc: lhsT [128 = ch + 64*ph, j, pair i, cls(12, padded 16)]
    wfc_s = np.sign(np.asarray(inputs["wfc"], np.float32))  # [12, 65536]
    wr = wfc_s.reshape(12, 64, 2, 2, 256)  # [cls, ch, ph, i, j]
    wl = np.zeros((128, 256, 2, 16), NP_FP8)
    for ph in range(2):
        # wl[ch + 64*ph, j, i, cls]
        wl[64 * ph:64 * ph + 64, :, :, 0:12] = \
            wr[:, :, ph, :, :].transpose(1, 3, 2, 0).astype(NP_FP8)
    out["wfc_l"] = wl
    bfc = np.asarray(inputs["bfc"], np.float32)
    out["bfc_t"] = bfc.reshape(12, 1).astype(np.float32)
    return out


def _make_ds(xs):
    """Host staging: exact 3-way bf16 split of x, parity-rearranged to
    ds[pair, h, (px, b, s, w')]. A lossless re-encoding (s0+s1+s2 == x in
    f32), so the device sums are bit-identical to summing x directly."""
    x = np.asarray(xs, np.float32).reshape(NPAIR, 2, 128, 128)
    s0 = x.astype(NP_BF16)
    r = x - s0.astype(np.float32)
    s1 = r.astype(NP_BF16)
    s2 = (r - s1.astype(np.float32)).astype(NP_BF16)
    sp = np.stack([s0, s1, s2], axis=0)        # [s, pair, b, h, w]
    sp = sp.reshape(3, NPAIR, 2, 128, 64, 2)   # [s, pair, b, h, w', px]
    ds = sp.transpose(1, 3, 2, 5, 0, 4)        # [pair, h, b, px, s, w']
    return np.ascontiguousarray(ds).reshape(NPAIR, 128, 768)


def kernel(**inputs):
    if "nc" not in _NC_CACHE:
        _NC_CACHE["nc"] = _build_nc()
    nc = _NC_CACHE["nc"]
    const = _prep_const_inputs(inputs)
    x = np.asarray(inputs["x"], np.float32)
    in_maps = []
    for c in range(NCORES):
        m = dict(const)
        m["ds"] = _make_ds(x[c * BPC:(c + 1) * BPC])
        in_maps.append(m)
    res = run_bass_kernel_spmd(nc, in_maps, core_ids=list(range(NCORES)))
    return np.concatenate([r["out"] for r in res.results], axis=0)
